# revision 12
# baseline (speedup 1.0000x reference)
"""Conv-QKV self-attention (CSA) Trainium2 Bass kernel.

Reference computation (per batch b):
    k = conv1d(x, K_w, K_b); q = conv1d(x, Q_w, Q_b); v = conv1d(x, V_w, V_b)
    scores = relu(k^T q)                # [L, L], contraction over 64 channels
    out = v @ scores / sqrt(3)          # [64, L], contraction over L

Sharding: 8 cores = 4 batches x 2 row-halves (l) of the score matrix.
Each core computes k, vT for its l-half, q for the full L, a flash-style
pass over relu(k^T q) tiles, and a PARTIAL out (contraction over its
l-half).  The host sums the two partials per batch.  1/sqrt(3) is folded
into the V weights on the host.

PE techniques:
 - row-packed score matmuls: two concurrent 64-row-group matmuls on
   duplicated k/q partitions (f32r streams 2 cyc/row, so a packed pair
   runs 2x the unpacked rate);
 - K-stacked convs: the host ships x with a 1-shifted duplicate on
   partitions 64:128, so taps 0+1 fuse into one K=128 matmul (2 matmuls
   per conv tile instead of 3);
 - software pipelining: the output matmul of iteration i issues after
   the score matmul of iteration i+1, hiding the relu PSUM->SBUF copy.
"""

import numpy as np

FIN, FOUT, KS = 64, 64, 3
B, L = 4, 4096
HALF = L // 2            # per-core l range
NCORES = 8
MT = 512                 # m tile (PSUM bank free dim, fp32)
LT = 128                 # l tile (PE partition dim)
N_MT = L // MT           # 8  (full m range per core)
N_LT = HALF // LT        # 16 (l tiles in this core's half)
SQRT_KS = float(np.sqrt(KS))

# matmul input dtype: "f32r" (2 cyc/row, ~tf32 precision), "bf16", or "f32"
MM_DTYPE = "f32r"

_NC_CACHE = {}


def _build_nc():
    from contextlib import ExitStack

    import concourse.tile as tile
    from concourse import bacc, mybir

    f32 = mybir.dt.float32
    mm_dt = {
        "f32r": mybir.dt.float32r,
        "bf16": mybir.dt.bfloat16,
        "f32": mybir.dt.float32,
    }[MM_DTYPE]
    AF = mybir.ActivationFunctionType

    nc = bacc.Bacc("TRN2", target_bir_lowering=False)

    # xk2: this core's l-window of x (+/-1 halo), with a 1-shifted copy
    # on partitions 64:128 (for K-stacked taps).  xd2: same for full x.
    xk_d = nc.dram_tensor("xk", [128, HALF + 2], f32, kind="ExternalInput")
    xd_d = nc.dram_tensor("xd", [128, L + 2], f32, kind="ExternalInput")
    # conv weights as [128, 2*FOUT]: cols 0:64 = [w_t0; w_t1] stacked on
    # partitions, cols 64:128 = [w_t2; 0]
    kw_d = nc.dram_tensor("kw", [128, 2 * FOUT], f32, kind="ExternalInput")
    qw_d = nc.dram_tensor("qw", [128, 2 * FOUT], f32, kind="ExternalInput")
    vw_d = nc.dram_tensor("vw", [KS, FIN, FOUT], f32, kind="ExternalInput")
    kb_d = nc.dram_tensor("kb", [FOUT, 1], f32, kind="ExternalInput")
    qb_d = nc.dram_tensor("qb", [FOUT, 1], f32, kind="ExternalInput")
    vb_d = nc.dram_tensor("vb", [1, FOUT], f32, kind="ExternalInput")
    out_d = nc.dram_tensor("out", [FOUT, L], f32, kind="ExternalOutput")

    NCH_K = 2   # DMA/round chunks for xk
    NCH_D = 4   # DMA/round chunks for xd

    with tile.TileContext(nc) as tc, ExitStack() as ctx:
        consts = ctx.enter_context(tc.tile_pool(name="consts", bufs=1))
        big = ctx.enter_context(tc.tile_pool(name="big", bufs=1))

        # chunked loads + f32r/bf16 rounding so the convs start early
        xk_f = consts.tile([128, HALF + 2], f32)
        xk_sb = consts.tile([128, HALF + 2], mm_dt)
        ck = (HALF + 2) // NCH_K
        for c in range(NCH_K):
            sl = slice(c * ck, (c + 1) * ck if c < NCH_K - 1 else HALF + 2)
            nc.sync.dma_start(out=xk_f[:, sl], in_=xk_d[:, sl])
            nc.vector.tensor_copy(xk_sb[:, sl], xk_f[:, sl])

        kw_f = consts.tile([128, 2 * FOUT], f32)
        nc.sync.dma_start(out=kw_f, in_=kw_d[:, :])
        qw_f = consts.tile([128, 2 * FOUT], f32)
        nc.sync.dma_start(out=qw_f, in_=qw_d[:, :])
        vw_f = consts.tile([FIN, KS, FOUT], f32)
        nc.sync.dma_start(out=vw_f, in_=vw_d[:, :, :].rearrange("t p c -> p t c"))
        kb_sb = consts.tile([FOUT, 1], f32)
        nc.sync.dma_start(out=kb_sb, in_=kb_d[:, :])
        qb_sb = consts.tile([FOUT, 1], f32)
        nc.sync.dma_start(out=qb_sb, in_=qb_d[:, :])
        vb_sb = consts.tile([128, FOUT], f32)
        nc.sync.dma_start(out=vb_sb, in_=vb_d[:, :].to_broadcast([128, FOUT]))

        kw_sb = consts.tile([128, 2 * FOUT], mm_dt)
        nc.scalar.copy(kw_sb, kw_f)
        qw_sb = consts.tile([128, 2 * FOUT], mm_dt)
        nc.scalar.copy(qw_sb, qw_f)
        vw_sb = consts.tile([FIN, KS, FOUT], mm_dt)
        nc.scalar.copy(vw_sb, vw_f)

        xd_f = consts.tile([128, L + 2], f32)
        xd_sb = consts.tile([128, L + 2], mm_dt)
        cd = (L + 2) // NCH_D
        for c in range(NCH_D):
            sl = slice(c * cd, (c + 1) * cd if c < NCH_D - 1 else L + 2)
            nc.gpsimd.dma_start(out=xd_f[:, sl], in_=xd_d[:, sl])
            nc.vector.tensor_copy(xd_sb[:, sl], xd_f[:, sl])

        # k (this half) and q (full), duplicated across both partition
        # halves for the row-packed score matmuls
        k2_sb = big.tile([128, HALF], mm_dt)
        q2_sb = big.tile([128, L], mm_dt)
        vt_sb = big.tile([128, N_LT, FOUT], mm_dt)

        # ---- stage A: conv projections -------------------------------
        actx = ctx.enter_context(ExitStack())
        cpool = actx.enter_context(tc.tile_pool(name="cpsum", bufs=2, space="PSUM"))

        def conv_kq(n_groups, x2, w2, bias, dst):
            for g in range(n_groups):
                p = cpool.tile([FOUT, MT], f32, name="pkq", tag="pkq")
                # taps 0+1 fused (K=128 over the shifted duplicate)
                nc.tensor.matmul(
                    p,
                    w2[:, 0:FOUT],
                    x2[:, g * MT : g * MT + MT],
                    start=True,
                    stop=False,
                )
                nc.tensor.matmul(
                    p,
                    w2[0:FIN, FOUT : 2 * FOUT],
                    x2[0:FIN, g * MT + 2 : g * MT + 2 + MT],
                    start=False,
                    stop=True,
                )
                nc.scalar.activation(
                    dst[0:FOUT, g * MT : (g + 1) * MT], p, AF.Identity, bias=bias
                )

        conv_kq(HALF // MT, xk_sb, kw_sb, kb_sb, k2_sb)
        conv_kq(L // MT, xd_sb, qw_sb, qb_sb, q2_sb)

        for j in range(N_LT):             # vT: [HALF, 64] in 128-row tiles
            pv = cpool.tile([128, FOUT], f32, name="pv", tag="pv")
            for t in range(KS):
                nc.tensor.matmul(
                    pv,
                    xk_sb[0:FIN, j * LT + t : j * LT + t + LT],
                    vw_sb[:, t, :],
                    start=(t == 0),
                    stop=(t == KS - 1),
                )
            nc.vector.tensor_add(vt_sb[:, j, :], pv, vb_sb)

        # duplicate k and q into partitions 64:128 (DMA, off engines)
        nc.sync.dma_start(out=k2_sb[FOUT:128, :], in_=k2_sb[0:FOUT, :])
        nc.gpsimd.dma_start(out=q2_sb[FOUT:128, :], in_=q2_sb[0:FOUT, :])

        # ---- stage B: flash loop over score tiles --------------------
        actx.close()
        spsum = ctx.enter_context(tc.tile_pool(name="spsum", bufs=2, space="PSUM"))
        spool = ctx.enter_context(tc.tile_pool(name="spool", bufs=3))
        opsum = ctx.enter_context(tc.tile_pool(name="opsum", bufs=2, space="PSUM"))
        opool = ctx.enter_context(tc.tile_pool(name="opool", bufs=2))

        for mp in range(N_MT // 2):
            mtA, mtB = 2 * mp, 2 * mp + 1
            poA = opsum.tile([FOUT, MT], f32, name="poA", tag="poA")
            poB = opsum.tile([FOUT, MT], f32, name="poB", tag="poB")
            pending = None  # (lj, s_sb) awaiting its mm2 pair
            for lj in range(N_LT):
                ps = spsum.tile([128, 2 * MT], f32, name="ps")
                # mm1 pair: row-packed (K=64 each) on duplicated k/q
                nc.tensor.matmul(
                    ps[:, 0:MT],
                    k2_sb[0:FOUT, lj * LT : (lj + 1) * LT],
                    q2_sb[0:FOUT, mtA * MT : (mtA + 1) * MT],
                    start=True,
                    stop=True,
                    tile_position=(0, 0),
                )
                nc.tensor.matmul(
                    ps[:, MT : 2 * MT],
                    k2_sb[FOUT:128, lj * LT : (lj + 1) * LT],
                    q2_sb[FOUT:128, mtB * MT : (mtB + 1) * MT],
                    start=True,
                    stop=True,
                    tile_position=(64, 0),
                )
                # software pipeline: issue previous iteration's mm2 pair
                # now, so the PE isn't blocked on this iteration's relu
                if pending is not None:
                    plj, ps_sb = pending
                    nc.tensor.matmul(
                        poA,
                        vt_sb[:, plj, :],
                        ps_sb[:, 0:MT],
                        start=(plj == 0),
                        stop=False,
                    )
                    nc.tensor.matmul(
                        poB,
                        vt_sb[:, plj, :],
                        ps_sb[:, MT : 2 * MT],
                        start=(plj == 0),
                        stop=False,
                    )
                s_sb = spool.tile([128, 2 * MT], mm_dt, name="s_sb")
                if lj % 2 == 0:
                    nc.vector.tensor_scalar_max(s_sb, ps, 0.0)
                else:
                    nc.scalar.activation(s_sb, ps, AF.Relu)
                pending = (lj, s_sb)

            plj, ps_sb = pending
            nc.tensor.matmul(
                poA, vt_sb[:, plj, :], ps_sb[:, 0:MT], start=(plj == 0), stop=True
            )
            nc.tensor.matmul(
                poB,
                vt_sb[:, plj, :],
                ps_sb[:, MT : 2 * MT],
                start=(plj == 0),
                stop=True,
            )
            o_sbA = opool.tile([FOUT, MT], f32, name="o_sbA", tag="oA")
            nc.scalar.copy(o_sbA, poA)
            nc.sync.dma_start(out_d[:, mtA * MT : (mtA + 1) * MT], o_sbA)
            o_sbB = opool.tile([FOUT, MT], f32, name="o_sbB", tag="oB")
            nc.scalar.copy(o_sbB, poB)
            nc.sync.dma_start(out_d[:, mtB * MT : (mtB + 1) * MT], o_sbB)

    nc.finalize()
    return nc


def _get_nc():
    if "nc" not in _NC_CACHE:
        _NC_CACHE["nc"] = _build_nc()
    return _NC_CACHE["nc"]


def make_in_maps(x, K_w, K_b, Q_w, Q_b, V_w, V_b):
    """Host-side marshalling: per-core input dicts for the SPMD kernel."""
    x = np.asarray(x, np.float32)
    # xpad col c = x col (c-1); cols 0, L+1, L+2 are zero
    xpad = np.zeros((B, FIN, L + 3), np.float32)
    xpad[:, :, 1 : L + 1] = x

    def wT(w):  # [co, ci, t] -> per-tap [ci, co]
        a = np.transpose(np.asarray(w, np.float32), (2, 1, 0))
        return a[0], a[1], a[2]

    def wstack(w):
        t0, t1, t2 = wT(w)
        s = np.zeros((128, 2 * FOUT), np.float32)
        s[0:FIN, 0:FOUT] = t0
        s[FIN:128, 0:FOUT] = t1
        s[0:FIN, FOUT : 2 * FOUT] = t2
        return s

    kw = wstack(K_w)
    qw = wstack(Q_w)
    vw = (
        np.ascontiguousarray(np.transpose(np.asarray(V_w, np.float32), (2, 1, 0)))
        / SQRT_KS
    )
    kb = np.asarray(K_b, np.float32).reshape(FOUT, 1)
    qb = np.asarray(Q_b, np.float32).reshape(FOUT, 1)
    vb = (np.asarray(V_b, np.float32) / SQRT_KS).reshape(1, FOUT)

    def shift_stack(a, lo, n):  # [64, n] window + 1-shifted copy
        return np.concatenate(
            [a[:, lo : lo + n], a[:, lo + 1 : lo + n + 1]], axis=0
        )

    in_maps = []
    for core in range(NCORES):
        b, h = divmod(core, 2)
        l0 = h * HALF
        xk = np.ascontiguousarray(shift_stack(xpad[b], l0, HALF + 2))
        xd = np.ascontiguousarray(shift_stack(xpad[b], 0, L + 2))
        in_maps.append(
            dict(xk=xk, xd=xd, kw=kw, qw=qw, vw=vw, kb=kb, qb=qb, vb=vb)
        )
    return in_maps


def assemble(results):
    out = np.empty((B, FOUT, L), np.float32)
    for b in range(B):
        out[b] = results[2 * b]["out"] + results[2 * b + 1]["out"]
    return out


def kernel(x, K_w, K_b, Q_w, Q_b, V_w, V_b):
    from concourse.bass_utils import run_bass_kernel_spmd

    nc = _get_nc()
    in_maps = make_in_maps(x, K_w, K_b, Q_w, Q_b, V_w, V_b)
    res = run_bass_kernel_spmd(nc, in_maps, core_ids=list(range(NCORES)))
    return assemble(res.results)


# revision 15
# speedup vs baseline: 1.1099x; 1.1099x over previous
"""Conv-QKV self-attention (CSA) Trainium2 Bass kernel.

Reference computation (per batch b):
    k = conv1d(x, K_w, K_b); q = conv1d(x, Q_w, Q_b); v = conv1d(x, V_w, V_b)
    scores = relu(k^T q)                # [L, L], contraction over 64 channels
    out = v @ scores / sqrt(3)          # [64, L], contraction over L

Sharding: 8 cores = 4 batches x 2 row-halves (l) of the score matrix.
Each core computes k, vT for its l-half, q for the full L, a flash-style
pass over relu(k^T q) tiles, and a PARTIAL out (contraction over its
l-half).  The host sums the two partials per batch.  1/sqrt(3) is folded
into the V weights on the host.

PE techniques:
 - row-packed score matmuls: two concurrent 64-row-group matmuls on
   duplicated k/q partitions (f32r streams 2 cyc/row, so a packed pair
   runs 2x the unpacked rate);
 - K-stacked convs: the host ships x with a 1-shifted duplicate on
   partitions 64:128, so taps 0+1 fuse into one K=128 matmul (2 matmuls
   per conv tile instead of 3);
 - software pipelining: the output matmul of iteration i issues after
   the score matmul of iteration i+1, hiding the relu PSUM->SBUF copy.
"""

import numpy as np

FIN, FOUT, KS = 64, 64, 3
B, L = 4, 4096
HALF = L // 2            # per-core l range
NCORES = 8
MT = 512                 # m tile (PSUM bank free dim, fp32)
LT = 128                 # l tile (PE partition dim)
N_MT = L // MT           # 8  (full m range per core)
N_LT = HALF // LT        # 16 (l tiles in this core's half)
SQRT_KS = float(np.sqrt(KS))

# matmul input dtype: "f32r" (2 cyc/row, ~tf32 precision), "bf16", or "f32"
MM_DTYPE = "f32r"

_NC_CACHE = {}


def _build_nc():
    from contextlib import ExitStack

    import concourse.tile as tile
    from concourse import bacc, mybir

    f32 = mybir.dt.float32
    mm_dt = {
        "f32r": mybir.dt.float32r,
        "bf16": mybir.dt.bfloat16,
        "f32": mybir.dt.float32,
    }[MM_DTYPE]
    AF = mybir.ActivationFunctionType

    nc = bacc.Bacc("TRN2", target_bir_lowering=False)

    # xk2: this core's l-window of x (+/-1 halo), with a 1-shifted copy
    # on partitions 64:128 (for K-stacked taps).  xd2: same for full x.
    xk_d = nc.dram_tensor("xk", [128, HALF + 2], f32, kind="ExternalInput")
    xd_d = nc.dram_tensor("xd", [128, L + 2], f32, kind="ExternalInput")
    # conv weights as [128, 2*FOUT]: cols 0:64 = [w_t0; w_t1] stacked on
    # partitions, cols 64:128 = [w_t2; 0]
    kw_d = nc.dram_tensor("kw", [128, 2 * FOUT], f32, kind="ExternalInput")
    qw_d = nc.dram_tensor("qw", [128, 2 * FOUT], f32, kind="ExternalInput")
    vw_d = nc.dram_tensor("vw", [KS, FIN, FOUT], f32, kind="ExternalInput")
    kb_d = nc.dram_tensor("kb", [FOUT, 1], f32, kind="ExternalInput")
    qb_d = nc.dram_tensor("qb", [FOUT, 1], f32, kind="ExternalInput")
    vb_d = nc.dram_tensor("vb", [1, FOUT], f32, kind="ExternalInput")
    out_d = nc.dram_tensor("out", [FOUT, L], f32, kind="ExternalOutput")

    NCH_K = 2   # DMA/round chunks for xk
    NCH_D = 4   # DMA/round chunks for xd

    with tile.TileContext(nc) as tc, ExitStack() as ctx:
        consts = ctx.enter_context(tc.tile_pool(name="consts", bufs=1))
        big = ctx.enter_context(tc.tile_pool(name="big", bufs=1))

        # weights + biases first (tiny; unblock the first conv matmuls)
        kw_f = consts.tile([128, 2 * FOUT], f32)
        nc.sync.dma_start(out=kw_f, in_=kw_d[:, :])
        qw_f = consts.tile([128, 2 * FOUT], f32)
        nc.sync.dma_start(out=qw_f, in_=qw_d[:, :])
        vw_f = consts.tile([FIN, KS, FOUT], f32)
        nc.sync.dma_start(out=vw_f, in_=vw_d[:, :, :].rearrange("t p c -> p t c"))
        kb_sb = consts.tile([FOUT, 1], f32)
        nc.sync.dma_start(out=kb_sb, in_=kb_d[:, :])
        qb_sb = consts.tile([FOUT, 1], f32)
        nc.sync.dma_start(out=qb_sb, in_=qb_d[:, :])
        vb_sb = consts.tile([128, FOUT], f32)
        nc.sync.dma_start(out=vb_sb, in_=vb_d[:, :].to_broadcast([128, FOUT]))

        kw_sb = consts.tile([128, 2 * FOUT], mm_dt)
        nc.scalar.copy(kw_sb, kw_f)
        qw_sb = consts.tile([128, 2 * FOUT], mm_dt)
        nc.scalar.copy(qw_sb, qw_f)
        vw_sb = consts.tile([FIN, KS, FOUT], mm_dt)
        nc.scalar.copy(vw_sb, vw_f)

        # chunked loads + f32r/bf16 rounding so the convs start early
        xk_f = consts.tile([128, HALF + 2], f32)
        xk_sb = consts.tile([128, HALF + 2], mm_dt)
        ck = (HALF + 2) // NCH_K
        for c in range(NCH_K):
            sl = slice(c * ck, (c + 1) * ck if c < NCH_K - 1 else HALF + 2)
            nc.sync.dma_start(out=xk_f[:, sl], in_=xk_d[:, sl])
            nc.vector.tensor_copy(xk_sb[:, sl], xk_f[:, sl])

        xd_f = consts.tile([128, L + 2], f32)
        xd_sb = consts.tile([128, L + 2], mm_dt)
        cd = (L + 2) // NCH_D
        for c in range(NCH_D):
            sl = slice(c * cd, (c + 1) * cd if c < NCH_D - 1 else L + 2)
            nc.gpsimd.dma_start(out=xd_f[:, sl], in_=xd_d[:, sl])
            nc.vector.tensor_copy(xd_sb[:, sl], xd_f[:, sl])

        # k (this half) and q (full), duplicated across both partition
        # halves for the row-packed score matmuls
        k2_sb = big.tile([128, HALF], mm_dt)
        q2_sb = big.tile([128, L], mm_dt)
        vt_sb = big.tile([128, N_LT, FOUT], mm_dt)

        # ---- stage A: conv projections -------------------------------
        actx = ctx.enter_context(ExitStack())
        cpool = actx.enter_context(tc.tile_pool(name="cpsum", bufs=2, space="PSUM"))

        def conv_kq(n_groups, x2, w2, bias, dst):
            # group pairs interleaved so consecutive PE matmuls hit
            # different PSUM banks (same-bank back-to-back matmuls lose
            # the fill/drain overlap)
            for gp in range(n_groups // 2):
                gA, gB = 2 * gp, 2 * gp + 1
                pA = cpool.tile([FOUT, MT], f32, name="pkqA", tag="pkqA")
                pB = cpool.tile([FOUT, MT], f32, name="pkqB", tag="pkqB")
                for g, p in ((gA, pA), (gB, pB)):
                    nc.tensor.matmul(
                        p,
                        w2[:, 0:FOUT],
                        x2[:, g * MT : g * MT + MT],
                        start=True,
                        stop=False,
                    )
                for g, p in ((gA, pA), (gB, pB)):
                    nc.tensor.matmul(
                        p,
                        w2[0:FIN, FOUT : 2 * FOUT],
                        x2[0:FIN, g * MT + 2 : g * MT + 2 + MT],
                        start=False,
                        stop=True,
                    )
                for g, p in ((gA, pA), (gB, pB)):
                    nc.scalar.activation(
                        dst[0:FOUT, g * MT : (g + 1) * MT], p, AF.Identity, bias=bias
                    )

        conv_kq(HALF // MT, xk_sb, kw_sb, kb_sb, k2_sb)
        conv_kq(L // MT, xd_sb, qw_sb, qb_sb, q2_sb)

        for j in range(N_LT):             # vT: [HALF, 64] in 128-row tiles
            pv = cpool.tile([128, FOUT], f32, name="pv", tag="pv")
            for t in range(KS):
                nc.tensor.matmul(
                    pv,
                    xk_sb[0:FIN, j * LT + t : j * LT + t + LT],
                    vw_sb[:, t, :],
                    start=(t == 0),
                    stop=(t == KS - 1),
                )
            nc.vector.tensor_add(vt_sb[:, j, :], pv, vb_sb)

        # duplicate k and q into partitions 64:128 (DMA, off engines)
        nc.sync.dma_start(out=k2_sb[FOUT:128, :], in_=k2_sb[0:FOUT, :])
        nc.gpsimd.dma_start(out=q2_sb[FOUT:128, :], in_=q2_sb[0:FOUT, :])

        # ---- stage B: flash loop over score tiles --------------------
        actx.close()
        spsum = ctx.enter_context(tc.tile_pool(name="spsum", bufs=2, space="PSUM"))
        spool = ctx.enter_context(tc.tile_pool(name="spool", bufs=4))
        opsum = ctx.enter_context(tc.tile_pool(name="opsum", bufs=2, space="PSUM"))
        opool = ctx.enter_context(tc.tile_pool(name="opool", bufs=2))

        for mp in range(N_MT // 2):
            mtA, mtB = 2 * mp, 2 * mp + 1
            poA = opsum.tile([FOUT, MT], f32, name="poA", tag="poA")
            poB = opsum.tile([FOUT, MT], f32, name="poB", tag="poB")
            pending = []  # [(lj, s_sb), ...] awaiting their mm2 pairs

            def flush_mm2(last=False):
                plj, ps_sb = pending.pop(0)
                nc.tensor.matmul(
                    poA,
                    vt_sb[:, plj, :],
                    ps_sb[:, 0:MT],
                    start=(plj == 0),
                    stop=last,
                )
                nc.tensor.matmul(
                    poB,
                    vt_sb[:, plj, :],
                    ps_sb[:, MT : 2 * MT],
                    start=(plj == 0),
                    stop=last,
                )

            for lj in range(N_LT):
                ps = spsum.tile([128, 2 * MT], f32, name="ps")
                # mm1 pair: row-packed (K=64 each) on duplicated k/q
                nc.tensor.matmul(
                    ps[:, 0:MT],
                    k2_sb[0:FOUT, lj * LT : (lj + 1) * LT],
                    q2_sb[0:FOUT, mtA * MT : (mtA + 1) * MT],
                    start=True,
                    stop=True,
                    tile_position=(0, 0),
                )
                nc.tensor.matmul(
                    ps[:, MT : 2 * MT],
                    k2_sb[FOUT:128, lj * LT : (lj + 1) * LT],
                    q2_sb[FOUT:128, mtB * MT : (mtB + 1) * MT],
                    start=True,
                    stop=True,
                    tile_position=(64, 0),
                )
                # software pipeline (depth 2): issue the mm2 pair from two
                # iterations ago, so the PE never blocks on a fresh relu
                if len(pending) >= 2:
                    flush_mm2()
                s_sb = spool.tile([128, 2 * MT], mm_dt, name="s_sb")
                if lj % 2 == 0:
                    nc.vector.tensor_scalar_max(s_sb, ps, 0.0)
                else:
                    nc.scalar.activation(s_sb, ps, AF.Relu)
                pending.append((lj, s_sb))

            while pending:
                flush_mm2(last=(len(pending) == 1))
            o_sbA = opool.tile([FOUT, MT], f32, name="o_sbA", tag="oA")
            nc.scalar.copy(o_sbA, poA)
            nc.sync.dma_start(out_d[:, mtA * MT : (mtA + 1) * MT], o_sbA)
            o_sbB = opool.tile([FOUT, MT], f32, name="o_sbB", tag="oB")
            nc.scalar.copy(o_sbB, poB)
            nc.sync.dma_start(out_d[:, mtB * MT : (mtB + 1) * MT], o_sbB)

    nc.finalize()
    return nc


def _get_nc():
    if "nc" not in _NC_CACHE:
        _NC_CACHE["nc"] = _build_nc()
    return _NC_CACHE["nc"]


def make_in_maps(x, K_w, K_b, Q_w, Q_b, V_w, V_b):
    """Host-side marshalling: per-core input dicts for the SPMD kernel."""
    x = np.asarray(x, np.float32)
    # xpad col c = x col (c-1); cols 0, L+1, L+2 are zero
    xpad = np.zeros((B, FIN, L + 3), np.float32)
    xpad[:, :, 1 : L + 1] = x

    def wT(w):  # [co, ci, t] -> per-tap [ci, co]
        a = np.transpose(np.asarray(w, np.float32), (2, 1, 0))
        return a[0], a[1], a[2]

    def wstack(w):
        t0, t1, t2 = wT(w)
        s = np.zeros((128, 2 * FOUT), np.float32)
        s[0:FIN, 0:FOUT] = t0
        s[FIN:128, 0:FOUT] = t1
        s[0:FIN, FOUT : 2 * FOUT] = t2
        return s

    kw = wstack(K_w)
    qw = wstack(Q_w)
    vw = (
        np.ascontiguousarray(np.transpose(np.asarray(V_w, np.float32), (2, 1, 0)))
        / SQRT_KS
    )
    kb = np.asarray(K_b, np.float32).reshape(FOUT, 1)
    qb = np.asarray(Q_b, np.float32).reshape(FOUT, 1)
    vb = (np.asarray(V_b, np.float32) / SQRT_KS).reshape(1, FOUT)

    def shift_stack(a, lo, n):  # [64, n] window + 1-shifted copy
        return np.concatenate(
            [a[:, lo : lo + n], a[:, lo + 1 : lo + n + 1]], axis=0
        )

    in_maps = []
    for core in range(NCORES):
        b, h = divmod(core, 2)
        l0 = h * HALF
        xk = np.ascontiguousarray(shift_stack(xpad[b], l0, HALF + 2))
        xd = np.ascontiguousarray(shift_stack(xpad[b], 0, L + 2))
        in_maps.append(
            dict(xk=xk, xd=xd, kw=kw, qw=qw, vw=vw, kb=kb, qb=qb, vb=vb)
        )
    return in_maps


def assemble(results):
    out = np.empty((B, FOUT, L), np.float32)
    for b in range(B):
        out[b] = results[2 * b]["out"] + results[2 * b + 1]["out"]
    return out


def kernel(x, K_w, K_b, Q_w, Q_b, V_w, V_b):
    from concourse.bass_utils import run_bass_kernel_spmd

    nc = _get_nc()
    in_maps = make_in_maps(x, K_w, K_b, Q_w, Q_b, V_w, V_b)
    res = run_bass_kernel_spmd(nc, in_maps, core_ids=list(range(NCORES)))
    return assemble(res.results)


# revision 19
# speedup vs baseline: 1.1456x; 1.0322x over previous
"""Conv-QKV self-attention (CSA) Trainium2 Bass kernel.

Reference computation (per batch b):
    k = conv1d(x, K_w, K_b); q = conv1d(x, Q_w, Q_b); v = conv1d(x, V_w, V_b)
    scores = relu(k^T q)                # [L, L], contraction over 64 channels
    out = v @ scores / sqrt(3)          # [64, L], contraction over L

Sharding: 8 cores = 4 batches x 2 row-halves (l) of the score matrix.
Each core computes k, vT for its l-half, q for the full L, a flash-style
pass over relu(k^T q) tiles, and a PARTIAL out (contraction over its
l-half).  The host sums the two partials per batch.  1/sqrt(3) is folded
into the V weights on the host.

PE techniques:
 - row-packed score matmuls: two concurrent 64-row-group matmuls on
   duplicated k/q partitions (f32r streams 2 cyc/row, so a packed pair
   runs 2x the unpacked rate);
 - K-stacked convs: the host ships x with a 1-shifted duplicate on
   partitions 64:128, so taps 0+1 fuse into one K=128 matmul (2 matmuls
   per conv tile instead of 3);
 - software pipelining: the output matmul of iteration i issues after
   the score matmul of iteration i+1, hiding the relu PSUM->SBUF copy.
"""

import numpy as np

FIN, FOUT, KS = 64, 64, 3
B, L = 4, 4096
HALF = L // 2            # per-core l range
NCORES = 8
MT = 512                 # m tile (PSUM bank free dim, fp32)
LT = 128                 # l tile (PE partition dim)
N_MT = L // MT           # 8  (full m range per core)
N_LT = HALF // LT        # 16 (l tiles in this core's half)
SQRT_KS = float(np.sqrt(KS))

# matmul input dtype: "f32r" (2 cyc/row, ~tf32 precision), "bf16", or "f32"
MM_DTYPE = "f32r"

_NC_CACHE = {}


def _build_nc():
    from contextlib import ExitStack

    import concourse.tile as tile
    from concourse import bacc, mybir

    f32 = mybir.dt.float32
    mm_dt = {
        "f32r": mybir.dt.float32r,
        "bf16": mybir.dt.bfloat16,
        "f32": mybir.dt.float32,
    }[MM_DTYPE]
    AF = mybir.ActivationFunctionType

    nc = bacc.Bacc("TRN2", target_bir_lowering=False)

    # xk2: this core's l-window of x (+/-1 halo), with a 1-shifted copy
    # on partitions 64:128 (for K-stacked taps).  xd2: same for full x.
    # Declared in the matmul dtype: the PE rounds f32r inputs on read, so
    # DMA-ing raw fp32 bits into an f32r tile is equivalent to rounding.
    xk_d = nc.dram_tensor("xk", [128, HALF + 2], mm_dt, kind="ExternalInput")
    xd_d = nc.dram_tensor("xd", [128, L + 2], mm_dt, kind="ExternalInput")
    # conv weights as [128, 2*FOUT]: cols 0:64 = [w_t0; w_t1] stacked on
    # partitions, cols 64:128 = [w_t2; 0]
    kw_d = nc.dram_tensor("kw", [128, 2 * FOUT], mm_dt, kind="ExternalInput")
    qw_d = nc.dram_tensor("qw", [128, 2 * FOUT], mm_dt, kind="ExternalInput")
    vw_d = nc.dram_tensor("vw", [KS, FIN, FOUT], mm_dt, kind="ExternalInput")
    kb_d = nc.dram_tensor("kb", [FOUT, 1], f32, kind="ExternalInput")
    qb_d = nc.dram_tensor("qb", [FOUT, 1], f32, kind="ExternalInput")
    vb_d = nc.dram_tensor("vb", [1, FOUT], f32, kind="ExternalInput")
    out_d = nc.dram_tensor("out", [FOUT, L], f32, kind="ExternalOutput")

    NCH_K = 2   # DMA/round chunks for xk
    NCH_D = 4   # DMA/round chunks for xd

    with tile.TileContext(nc) as tc, ExitStack() as ctx:
        consts = ctx.enter_context(tc.tile_pool(name="consts", bufs=1))
        big = ctx.enter_context(tc.tile_pool(name="big", bufs=1))

        # x windows first (gate the convs); chunked so the first conv
        # group can start as soon as its chunk lands
        xk_sb = consts.tile([128, HALF + 2], mm_dt)
        ck = (HALF + 2) // NCH_K
        for c in range(NCH_K):
            sl = slice(c * ck, (c + 1) * ck if c < NCH_K - 1 else HALF + 2)
            nc.sync.dma_start(out=xk_sb[:, sl], in_=xk_d[:, sl])
        xd_sb = consts.tile([128, L + 2], mm_dt)
        cd = (L + 2) // NCH_D
        for c in range(NCH_D):
            sl = slice(c * cd, (c + 1) * cd if c < NCH_D - 1 else L + 2)
            nc.gpsimd.dma_start(out=xd_sb[:, sl], in_=xd_d[:, sl])

        kw_sb = consts.tile([128, 2 * FOUT], mm_dt)
        nc.scalar.dma_start(out=kw_sb, in_=kw_d[:, :])
        qw_sb = consts.tile([128, 2 * FOUT], mm_dt)
        nc.scalar.dma_start(out=qw_sb, in_=qw_d[:, :])
        vw_sb = consts.tile([FIN, KS, FOUT], mm_dt)
        nc.scalar.dma_start(out=vw_sb, in_=vw_d[:, :, :].rearrange("t p c -> p t c"))
        kb_sb = consts.tile([FOUT, 1], f32)
        nc.scalar.dma_start(out=kb_sb, in_=kb_d[:, :])
        qb_sb = consts.tile([FOUT, 1], f32)
        nc.scalar.dma_start(out=qb_sb, in_=qb_d[:, :])
        vb_sb = consts.tile([128, FOUT], f32)
        nc.scalar.dma_start(out=vb_sb, in_=vb_d[:, :].to_broadcast([128, FOUT]))

        # k (this half) and q (full), duplicated across both partition
        # halves for the row-packed score matmuls
        k2_sb = big.tile([128, HALF], mm_dt)
        q2_sb = big.tile([128, L], mm_dt)
        vt_sb = big.tile([128, N_LT, FOUT], mm_dt)

        # ---- stage A: conv projections -------------------------------
        actx = ctx.enter_context(ExitStack())
        cpool = actx.enter_context(tc.tile_pool(name="cpsum", bufs=2, space="PSUM"))

        def conv_kq(n_groups, x2, w2, bias, dst):
            # group pairs interleaved so consecutive PE matmuls hit
            # different PSUM banks (same-bank back-to-back matmuls lose
            # the fill/drain overlap)
            for gp in range(n_groups // 2):
                gA, gB = 2 * gp, 2 * gp + 1
                pA = cpool.tile([FOUT, MT], f32, name="pkqA", tag="pkqA")
                pB = cpool.tile([FOUT, MT], f32, name="pkqB", tag="pkqB")
                for g, p in ((gA, pA), (gB, pB)):
                    nc.tensor.matmul(
                        p,
                        w2[:, 0:FOUT],
                        x2[:, g * MT : g * MT + MT],
                        start=True,
                        stop=False,
                    )
                for g, p in ((gA, pA), (gB, pB)):
                    nc.tensor.matmul(
                        p,
                        w2[0:FIN, FOUT : 2 * FOUT],
                        x2[0:FIN, g * MT + 2 : g * MT + 2 + MT],
                        start=False,
                        stop=True,
                    )
                for g, p in ((gA, pA), (gB, pB)):
                    nc.scalar.activation(
                        dst[0:FOUT, g * MT : (g + 1) * MT], p, AF.Identity, bias=bias
                    )
                # duplicate into partitions 64:128 for the row-packed
                # score matmuls (chunked: overlaps the remaining convs)
                dsl = slice(gA * MT, (gB + 1) * MT)
                nc.sync.dma_start(out=dst[FOUT:128, dsl], in_=dst[0:FOUT, dsl])

        conv_kq(HALF // MT, xk_sb, kw_sb, kb_sb, k2_sb)
        conv_kq(L // MT, xd_sb, qw_sb, qb_sb, q2_sb)

        for j in range(N_LT):             # vT: [HALF, 64] in 128-row tiles
            pv = cpool.tile([128, FOUT], f32, name="pv", tag="pv")
            for t in range(KS):
                nc.tensor.matmul(
                    pv,
                    xk_sb[0:FIN, j * LT + t : j * LT + t + LT],
                    vw_sb[:, t, :],
                    start=(t == 0),
                    stop=(t == KS - 1),
                )
            nc.vector.tensor_add(vt_sb[:, j, :], pv, vb_sb)

        # ---- stage B: flash loop over score tiles --------------------
        actx.close()
        spsum = ctx.enter_context(tc.tile_pool(name="spsum", bufs=2, space="PSUM"))
        spool = ctx.enter_context(tc.tile_pool(name="spool", bufs=4))
        opsum = ctx.enter_context(tc.tile_pool(name="opsum", bufs=2, space="PSUM"))
        opool = ctx.enter_context(tc.tile_pool(name="opool", bufs=2))

        for mp in range(N_MT // 2):
            mtA, mtB = 2 * mp, 2 * mp + 1
            poA = opsum.tile([FOUT, MT], f32, name="poA", tag="poA")
            poB = opsum.tile([FOUT, MT], f32, name="poB", tag="poB")
            pending = []  # [(lj, s_sb), ...] awaiting their mm2 pairs

            def flush_mm2(last=False):
                plj, ps_sb = pending.pop(0)
                nc.tensor.matmul(
                    poA,
                    vt_sb[:, plj, :],
                    ps_sb[:, 0:MT],
                    start=(plj == 0),
                    stop=last,
                )
                nc.tensor.matmul(
                    poB,
                    vt_sb[:, plj, :],
                    ps_sb[:, MT : 2 * MT],
                    start=(plj == 0),
                    stop=last,
                )

            for lj in range(N_LT):
                ps = spsum.tile([128, 2 * MT], f32, name="ps")
                # mm1 pair: row-packed (K=64 each) on duplicated k/q
                nc.tensor.matmul(
                    ps[:, 0:MT],
                    k2_sb[0:FOUT, lj * LT : (lj + 1) * LT],
                    q2_sb[0:FOUT, mtA * MT : (mtA + 1) * MT],
                    start=True,
                    stop=True,
                    tile_position=(0, 0),
                )
                nc.tensor.matmul(
                    ps[:, MT : 2 * MT],
                    k2_sb[FOUT:128, lj * LT : (lj + 1) * LT],
                    q2_sb[FOUT:128, mtB * MT : (mtB + 1) * MT],
                    start=True,
                    stop=True,
                    tile_position=(64, 0),
                )
                # software pipeline (depth 2): issue the mm2 pair from two
                # iterations ago, so the PE never blocks on a fresh relu
                if len(pending) >= 2:
                    flush_mm2()
                s_sb = spool.tile([128, 2 * MT], mm_dt, name="s_sb")
                if lj % 2 == 0:
                    nc.vector.tensor_scalar_max(s_sb, ps, 0.0)
                else:
                    nc.scalar.activation(s_sb, ps, AF.Relu)
                pending.append((lj, s_sb))

            while pending:
                flush_mm2(last=(len(pending) == 1))
            o_sbA = opool.tile([FOUT, MT], f32, name="o_sbA", tag="oA")
            nc.scalar.copy(o_sbA, poA)
            nc.sync.dma_start(out_d[:, mtA * MT : (mtA + 1) * MT], o_sbA)
            o_sbB = opool.tile([FOUT, MT], f32, name="o_sbB", tag="oB")
            nc.scalar.copy(o_sbB, poB)
            nc.sync.dma_start(out_d[:, mtB * MT : (mtB + 1) * MT], o_sbB)

    nc.finalize()
    return nc


def _get_nc():
    if "nc" not in _NC_CACHE:
        _NC_CACHE["nc"] = _build_nc()
    return _NC_CACHE["nc"]


def make_in_maps(x, K_w, K_b, Q_w, Q_b, V_w, V_b):
    """Host-side marshalling: per-core input dicts for the SPMD kernel."""
    x = np.asarray(x, np.float32)
    # xpad col c = x col (c-1); cols 0, L+1, L+2 are zero
    xpad = np.zeros((B, FIN, L + 3), np.float32)
    xpad[:, :, 1 : L + 1] = x

    def wT(w):  # [co, ci, t] -> per-tap [ci, co]
        a = np.transpose(np.asarray(w, np.float32), (2, 1, 0))
        return a[0], a[1], a[2]

    def wstack(w):
        t0, t1, t2 = wT(w)
        s = np.zeros((128, 2 * FOUT), np.float32)
        s[0:FIN, 0:FOUT] = t0
        s[FIN:128, 0:FOUT] = t1
        s[0:FIN, FOUT : 2 * FOUT] = t2
        return s

    kw = wstack(K_w)
    qw = wstack(Q_w)
    vw = (
        np.ascontiguousarray(np.transpose(np.asarray(V_w, np.float32), (2, 1, 0)))
        / SQRT_KS
    )
    kb = np.asarray(K_b, np.float32).reshape(FOUT, 1)
    qb = np.asarray(Q_b, np.float32).reshape(FOUT, 1)
    vb = (np.asarray(V_b, np.float32) / SQRT_KS).reshape(1, FOUT)

    def shift_stack(a, lo, n):  # [64, n] window + 1-shifted copy
        return np.concatenate(
            [a[:, lo : lo + n], a[:, lo + 1 : lo + n + 1]], axis=0
        )

    in_maps = []
    for core in range(NCORES):
        b, h = divmod(core, 2)
        l0 = h * HALF
        xk = np.ascontiguousarray(shift_stack(xpad[b], l0, HALF + 2))
        xd = np.ascontiguousarray(shift_stack(xpad[b], 0, L + 2))
        in_maps.append(
            dict(xk=xk, xd=xd, kw=kw, qw=qw, vw=vw, kb=kb, qb=qb, vb=vb)
        )
    return in_maps


def assemble(results):
    out = np.empty((B, FOUT, L), np.float32)
    for b in range(B):
        out[b] = results[2 * b]["out"] + results[2 * b + 1]["out"]
    return out


def kernel(x, K_w, K_b, Q_w, Q_b, V_w, V_b):
    from concourse.bass_utils import run_bass_kernel_spmd

    nc = _get_nc()
    in_maps = make_in_maps(x, K_w, K_b, Q_w, Q_b, V_w, V_b)
    res = run_bass_kernel_spmd(nc, in_maps, core_ids=list(range(NCORES)))
    return assemble(res.results)


# revision 20
# speedup vs baseline: 1.1732x; 1.0241x over previous
"""Conv-QKV self-attention (CSA) Trainium2 Bass kernel.

Reference computation (per batch b):
    k = conv1d(x, K_w, K_b); q = conv1d(x, Q_w, Q_b); v = conv1d(x, V_w, V_b)
    scores = relu(k^T q)                # [L, L], contraction over 64 channels
    out = v @ scores / sqrt(3)          # [64, L], contraction over L

Sharding: 8 cores = 4 batches x 2 row-halves (l) of the score matrix.
Each core computes k, vT for its l-half, q for the full L, a flash-style
pass over relu(k^T q) tiles, and a PARTIAL out (contraction over its
l-half).  The host sums the two partials per batch.  1/sqrt(3) is folded
into the V weights on the host.

PE techniques:
 - row-packed score matmuls: two concurrent 64-row-group matmuls on
   duplicated k/q partitions (f32r streams 2 cyc/row, so a packed pair
   runs 2x the unpacked rate);
 - K-stacked convs: the host ships x with a 1-shifted duplicate on
   partitions 64:128, so taps 0+1 fuse into one K=128 matmul (2 matmuls
   per conv tile instead of 3);
 - software pipelining: the output matmul of iteration i issues after
   the score matmul of iteration i+1, hiding the relu PSUM->SBUF copy.
"""

import numpy as np

FIN, FOUT, KS = 64, 64, 3
B, L = 4, 4096
HALF = L // 2            # per-core l range
NCORES = 8
MT = 512                 # m tile (PSUM bank free dim, fp32)
LT = 128                 # l tile (PE partition dim)
N_MT = L // MT           # 8  (full m range per core)
N_LT = HALF // LT        # 16 (l tiles in this core's half)
SQRT_KS = float(np.sqrt(KS))

# matmul input dtype: "f32r" (2 cyc/row, ~tf32 precision), "bf16", or "f32"
MM_DTYPE = "f32r"

_NC_CACHE = {}


def _build_nc():
    from contextlib import ExitStack

    import concourse.tile as tile
    from concourse import bacc, mybir

    f32 = mybir.dt.float32
    mm_dt = {
        "f32r": mybir.dt.float32r,
        "bf16": mybir.dt.bfloat16,
        "f32": mybir.dt.float32,
    }[MM_DTYPE]
    AF = mybir.ActivationFunctionType

    nc = bacc.Bacc("TRN2", target_bir_lowering=False)

    # xk2: this core's l-window of x (+/-1 halo), with a 1-shifted copy
    # on partitions 64:128 (for K-stacked taps).  xd2: same for full x.
    # Declared in the matmul dtype: the PE rounds f32r inputs on read, so
    # DMA-ing raw fp32 bits into an f32r tile is equivalent to rounding.
    xk_d = nc.dram_tensor("xk", [128, HALF + 2], mm_dt, kind="ExternalInput")
    xd_d = nc.dram_tensor("xd", [128, L + 2], mm_dt, kind="ExternalInput")
    # conv weights as [128, 2*FOUT]: cols 0:64 = [w_t0; w_t1] stacked on
    # partitions, cols 64:128 = [w_t2; 0]
    kw_d = nc.dram_tensor("kw", [128, 2 * FOUT], mm_dt, kind="ExternalInput")
    qw_d = nc.dram_tensor("qw", [128, 2 * FOUT], mm_dt, kind="ExternalInput")
    vw_d = nc.dram_tensor("vw", [KS, FIN, FOUT], mm_dt, kind="ExternalInput")
    kb_d = nc.dram_tensor("kb", [FOUT, 1], f32, kind="ExternalInput")
    qb_d = nc.dram_tensor("qb", [FOUT, 1], f32, kind="ExternalInput")
    vb_d = nc.dram_tensor("vb", [1, FOUT], f32, kind="ExternalInput")
    out_d = nc.dram_tensor("out", [FOUT, L], f32, kind="ExternalOutput")

    NCH_K = 2   # DMA/round chunks for xk
    NCH_D = 4   # DMA/round chunks for xd

    with tile.TileContext(nc) as tc, ExitStack() as ctx:
        consts = ctx.enter_context(tc.tile_pool(name="consts", bufs=1))
        big = ctx.enter_context(tc.tile_pool(name="big", bufs=1))

        # x windows first (gate the convs); chunked so the first conv
        # group can start as soon as its chunk lands
        xk_sb = consts.tile([128, HALF + 2], mm_dt)
        ck = (HALF + 2) // NCH_K
        for c in range(NCH_K):
            sl = slice(c * ck, (c + 1) * ck if c < NCH_K - 1 else HALF + 2)
            nc.sync.dma_start(out=xk_sb[:, sl], in_=xk_d[:, sl])
        kw_sb = consts.tile([128, 2 * FOUT], mm_dt)
        nc.gpsimd.dma_start(out=kw_sb, in_=kw_d[:, :])
        qw_sb = consts.tile([128, 2 * FOUT], mm_dt)
        nc.gpsimd.dma_start(out=qw_sb, in_=qw_d[:, :])
        vw_sb = consts.tile([FIN, KS, FOUT], mm_dt)
        nc.gpsimd.dma_start(out=vw_sb, in_=vw_d[:, :, :].rearrange("t p c -> p t c"))
        kb_sb = consts.tile([FOUT, 1], f32)
        nc.sync.dma_start(out=kb_sb, in_=kb_d[:, :])
        qb_sb = consts.tile([FOUT, 1], f32)
        nc.sync.dma_start(out=qb_sb, in_=qb_d[:, :])
        vb_sb = consts.tile([128, FOUT], f32)
        nc.sync.dma_start(out=vb_sb, in_=vb_d[:, :].to_broadcast([128, FOUT]))

        xd_sb = consts.tile([128, L + 2], mm_dt)
        cd = (L + 2) // NCH_D
        for c in range(NCH_D):
            sl = slice(c * cd, (c + 1) * cd if c < NCH_D - 1 else L + 2)
            nc.gpsimd.dma_start(out=xd_sb[:, sl], in_=xd_d[:, sl])

        # k (this half) and q (full), duplicated across both partition
        # halves for the row-packed score matmuls
        k2_sb = big.tile([128, HALF], mm_dt)
        q2_sb = big.tile([128, L], mm_dt)
        vt_sb = big.tile([128, N_LT, FOUT], mm_dt)

        # ---- stage A: conv projections -------------------------------
        actx = ctx.enter_context(ExitStack())
        cpool = actx.enter_context(tc.tile_pool(name="cpsum", bufs=2, space="PSUM"))

        def conv_kq(n_groups, x2, w2, bias, dst):
            # group pairs interleaved so consecutive PE matmuls hit
            # different PSUM banks (same-bank back-to-back matmuls lose
            # the fill/drain overlap)
            for gp in range(n_groups // 2):
                gA, gB = 2 * gp, 2 * gp + 1
                pA = cpool.tile([FOUT, MT], f32, name="pkqA", tag="pkqA")
                pB = cpool.tile([FOUT, MT], f32, name="pkqB", tag="pkqB")
                for g, p in ((gA, pA), (gB, pB)):
                    nc.tensor.matmul(
                        p,
                        w2[:, 0:FOUT],
                        x2[:, g * MT : g * MT + MT],
                        start=True,
                        stop=False,
                    )
                for g, p in ((gA, pA), (gB, pB)):
                    nc.tensor.matmul(
                        p,
                        w2[0:FIN, FOUT : 2 * FOUT],
                        x2[0:FIN, g * MT + 2 : g * MT + 2 + MT],
                        start=False,
                        stop=True,
                    )
                for g, p in ((gA, pA), (gB, pB)):
                    nc.scalar.activation(
                        dst[0:FOUT, g * MT : (g + 1) * MT], p, AF.Identity, bias=bias
                    )
                # duplicate into partitions 64:128 for the row-packed
                # score matmuls (chunked: overlaps the remaining convs)
                dsl = slice(gA * MT, (gB + 1) * MT)
                nc.sync.dma_start(out=dst[FOUT:128, dsl], in_=dst[0:FOUT, dsl])

        for j in range(N_LT):             # vT: [HALF, 64] in 128-row tiles
            pv = cpool.tile([128, FOUT], f32, name="pv", tag="pv")
            for t in range(KS):
                nc.tensor.matmul(
                    pv,
                    xk_sb[0:FIN, j * LT + t : j * LT + t + LT],
                    vw_sb[:, t, :],
                    start=(t == 0),
                    stop=(t == KS - 1),
                )
            nc.vector.tensor_add(vt_sb[:, j, :], pv, vb_sb)

        conv_kq(HALF // MT, xk_sb, kw_sb, kb_sb, k2_sb)
        conv_kq(L // MT, xd_sb, qw_sb, qb_sb, q2_sb)

        # ---- stage B: flash loop over score tiles --------------------
        actx.close()
        spsum = ctx.enter_context(tc.tile_pool(name="spsum", bufs=2, space="PSUM"))
        spool = ctx.enter_context(tc.tile_pool(name="spool", bufs=4))
        opsum = ctx.enter_context(tc.tile_pool(name="opsum", bufs=2, space="PSUM"))
        opool = ctx.enter_context(tc.tile_pool(name="opool", bufs=2))

        for mp in range(N_MT // 2):
            mtA, mtB = 2 * mp, 2 * mp + 1
            poA = opsum.tile([FOUT, MT], f32, name="poA", tag="poA")
            poB = opsum.tile([FOUT, MT], f32, name="poB", tag="poB")
            pending = []  # [(lj, s_sb), ...] awaiting their mm2 pairs

            def flush_mm2(last=False):
                plj, ps_sb = pending.pop(0)
                nc.tensor.matmul(
                    poA,
                    vt_sb[:, plj, :],
                    ps_sb[:, 0:MT],
                    start=(plj == 0),
                    stop=last,
                )
                nc.tensor.matmul(
                    poB,
                    vt_sb[:, plj, :],
                    ps_sb[:, MT : 2 * MT],
                    start=(plj == 0),
                    stop=last,
                )

            for lj in range(N_LT):
                ps = spsum.tile([128, 2 * MT], f32, name="ps")
                # mm1 pair: row-packed (K=64 each) on duplicated k/q
                nc.tensor.matmul(
                    ps[:, 0:MT],
                    k2_sb[0:FOUT, lj * LT : (lj + 1) * LT],
                    q2_sb[0:FOUT, mtA * MT : (mtA + 1) * MT],
                    start=True,
                    stop=True,
                    tile_position=(0, 0),
                )
                nc.tensor.matmul(
                    ps[:, MT : 2 * MT],
                    k2_sb[FOUT:128, lj * LT : (lj + 1) * LT],
                    q2_sb[FOUT:128, mtB * MT : (mtB + 1) * MT],
                    start=True,
                    stop=True,
                    tile_position=(64, 0),
                )
                # software pipeline (depth 2): issue the mm2 pair from two
                # iterations ago, so the PE never blocks on a fresh relu
                if len(pending) >= 2:
                    flush_mm2()
                s_sb = spool.tile([128, 2 * MT], mm_dt, name="s_sb")
                if lj % 2 == 0:
                    nc.vector.tensor_scalar_max(s_sb, ps, 0.0)
                else:
                    nc.scalar.activation(s_sb, ps, AF.Relu)
                pending.append((lj, s_sb))

            while pending:
                flush_mm2(last=(len(pending) == 1))
            o_sbA = opool.tile([FOUT, MT], f32, name="o_sbA", tag="oA")
            nc.scalar.copy(o_sbA, poA)
            nc.sync.dma_start(out_d[:, mtA * MT : (mtA + 1) * MT], o_sbA)
            o_sbB = opool.tile([FOUT, MT], f32, name="o_sbB", tag="oB")
            nc.scalar.copy(o_sbB, poB)
            nc.sync.dma_start(out_d[:, mtB * MT : (mtB + 1) * MT], o_sbB)

    nc.finalize()
    return nc


def _get_nc():
    if "nc" not in _NC_CACHE:
        _NC_CACHE["nc"] = _build_nc()
    return _NC_CACHE["nc"]


def make_in_maps(x, K_w, K_b, Q_w, Q_b, V_w, V_b):
    """Host-side marshalling: per-core input dicts for the SPMD kernel."""
    x = np.asarray(x, np.float32)
    # xpad col c = x col (c-1); cols 0, L+1, L+2 are zero
    xpad = np.zeros((B, FIN, L + 3), np.float32)
    xpad[:, :, 1 : L + 1] = x

    def wT(w):  # [co, ci, t] -> per-tap [ci, co]
        a = np.transpose(np.asarray(w, np.float32), (2, 1, 0))
        return a[0], a[1], a[2]

    def wstack(w):
        t0, t1, t2 = wT(w)
        s = np.zeros((128, 2 * FOUT), np.float32)
        s[0:FIN, 0:FOUT] = t0
        s[FIN:128, 0:FOUT] = t1
        s[0:FIN, FOUT : 2 * FOUT] = t2
        return s

    kw = wstack(K_w)
    qw = wstack(Q_w)
    vw = (
        np.ascontiguousarray(np.transpose(np.asarray(V_w, np.float32), (2, 1, 0)))
        / SQRT_KS
    )
    kb = np.asarray(K_b, np.float32).reshape(FOUT, 1)
    qb = np.asarray(Q_b, np.float32).reshape(FOUT, 1)
    vb = (np.asarray(V_b, np.float32) / SQRT_KS).reshape(1, FOUT)

    def shift_stack(a, lo, n):  # [64, n] window + 1-shifted copy
        return np.concatenate(
            [a[:, lo : lo + n], a[:, lo + 1 : lo + n + 1]], axis=0
        )

    in_maps = []
    for core in range(NCORES):
        b, h = divmod(core, 2)
        l0 = h * HALF
        xk = np.ascontiguousarray(shift_stack(xpad[b], l0, HALF + 2))
        xd = np.ascontiguousarray(shift_stack(xpad[b], 0, L + 2))
        in_maps.append(
            dict(xk=xk, xd=xd, kw=kw, qw=qw, vw=vw, kb=kb, qb=qb, vb=vb)
        )
    return in_maps


def assemble(results):
    out = np.empty((B, FOUT, L), np.float32)
    for b in range(B):
        out[b] = results[2 * b]["out"] + results[2 * b + 1]["out"]
    return out


def kernel(x, K_w, K_b, Q_w, Q_b, V_w, V_b):
    from concourse.bass_utils import run_bass_kernel_spmd

    nc = _get_nc()
    in_maps = make_in_maps(x, K_w, K_b, Q_w, Q_b, V_w, V_b)
    res = run_bass_kernel_spmd(nc, in_maps, core_ids=list(range(NCORES)))
    return assemble(res.results)
